# revision 11
# baseline (speedup 1.0000x reference)
"""GQA attention (SEQ=2048, DIM=4096, 32 Q heads / 8 KV heads, head_dim=128),
tensor-parallel over heads across 8 NeuronCores.

Each core owns 4 Q heads + 1 KV head: wq/wk/wv split column-wise, wo split
row-wise; each core produces a partial (2048, 4096) output (bf16) that the
host sums (the all-reduce of row-parallel wo).

Per-core kernel:
  A) QKV projections: stream xT (dim-major) blocks; Q^T/K^T/V^T accumulate in
     PSUM over the 4096 contraction; RoPE applied on PSUM eviction; V^T
     transposed back to V-natural (bf16, ones column appended) with XBAR DMA
     transposes (no PE time).  Startup DMAs are per-ktile across queues so
     matmul 0 starts early.  The last seq-block's accumulators are evicted to
     SBUF on DVE so phase B's PSUM pools free immediately; its RoPE (rotated
     halves fetched via DMA partition-swap) overlaps phase B.
  B) Attention per (query-block, head): S^T = K^T_blk.T @ Q^T (keys on
     partitions), exp on ACT (scale=1/sqrt(128) folded in) emitting bf16
     probs, causal triangle applied multiplicatively (0/1 bf16) on the
     diagonal 128-blocks after exp.  AV runs in natural-O orientation:
     lhsT = expS^T 128-col slice, rhs = [V_blk | 1] (129 cols) so the softmax
     denominator accumulates for free in column 128 of PSUM.  Eviction
     normalizes with a per-partition ACT scale (1/D) into bf16; O -> O^T via
     XBAR DMA transpose (no PE time).
  C) out = O^T.T @ wo (all bf16) accumulated over the 4 heads, streamed to
     DRAM as bf16 partials in 2048-wide stores.
"""

import numpy as np
import ml_dtypes

import concourse.bacc as bacc
import concourse.tile as tile
from concourse import mybir
from concourse.bass_utils import run_bass_kernel_spmd

F32 = mybir.dt.float32
BF16 = mybir.dt.bfloat16

DIM = 4096
SEQ = 2048
HEAD_DIM = 128
N_CORES = 8
QH = 4              # q heads per core
QS = QH * HEAD_DIM  # 512: wq column slice per core
NKT = DIM // 128    # 32 contraction tiles
NSB = SEQ // 512    # 4 sequence blocks
NKB = SEQ // 128    # 16 key blocks
SCALE = 1.0 / float(np.sqrt(HEAD_DIM))
LAG = 4             # AV matmuls trail the score stream by LAG key blocks


def build_nc():
    nc = bacc.Bacc(trn_type="TRN2")

    xT = nc.declare_dram_parameter("xT", [DIM, SEQ], BF16, isOutput=False)
    wq = nc.declare_dram_parameter("wq", [DIM, QS], BF16, isOutput=False)
    wk = nc.declare_dram_parameter("wk", [DIM, HEAD_DIM], BF16, isOutput=False)
    wv = nc.declare_dram_parameter("wv", [DIM, HEAD_DIM], BF16, isOutput=False)
    wo = nc.declare_dram_parameter("wo", [QS, DIM], BF16, isOutput=False)
    cosT = nc.declare_dram_parameter("cosT", [HEAD_DIM, SEQ], F32, isOutput=False)
    sinTs = nc.declare_dram_parameter("sinTs", [HEAD_DIM, SEQ], F32, isOutput=False)
    tri = nc.declare_dram_parameter("tri", [128, 128], BF16, isOutput=False)
    onesv = nc.declare_dram_parameter("onesv", [128, NKB, 1], BF16, isOutput=False)
    out = nc.declare_dram_parameter("out", [SEQ, DIM], BF16, isOutput=True)

    with tile.TileContext(nc) as tc:
        with (
            tc.tile_pool(name="persist", bufs=1) as persist,
            tc.tile_pool(name="resid", bufs=1) as resid,
            tc.tile_pool(name="vtbo", bufs=2) as vtbo,
            tc.tile_pool(name="cspool", bufs=2) as cspool,
            tc.tile_pool(name="ropetmp", bufs=2) as ropetmp,
            tc.tile_pool(name="wopool", bufs=1) as wopool,
            tc.tile_pool(name="expp", bufs=18) as expp,
            tc.tile_pool(name="dpool", bufs=6) as dpool,
            tc.tile_pool(name="ospool", bufs=6) as ospool,
            tc.tile_pool(name="otpool", bufs=2) as otpool,
            tc.tile_pool(name="outev", bufs=3) as outev,
        ):
            # resident activations (per-seq-block tiles so cross-phase
            # dependencies stay precise)
            qTs = [resid.tile([128, QH, 512], BF16, name=f"qT{sb}")
                   for sb in range(NSB)]
            kTs = [resid.tile([128, 512], BF16, name=f"kT{sb}")
                   for sb in range(NSB)]
            # V natural (keys, d) in bf16 with a ones column at 128
            # block pitch padded to 144 (16-elem granule) so XBAR DMA
            # transposes land 32B-aligned and never touch the ones column
            vext = resid.tile([128, NKB, 144], BF16)

            tri_sb = persist.tile([128, 128], BF16)
            # sb3 eviction staging (bf16): [k, q0..q3] straight + rotated
            ev3 = persist.tile([128, 5, 512], BF16)
            vr3 = persist.tile([128, 5, 512], BF16)

            rope3_box = []  # sb3 RoPE, emitted after qb0's attention
            wo_sb = wopool.tile([128, QH, DIM], BF16)
            wo_r = wo.rearrange("(h p) n -> p h n", p=128)

            # ---------------- Phase A: projections + RoPE ----------------
            with (
                tc.tile_pool(name="wpool", bufs=1) as wpool,
                tc.tile_pool(name="xpool", bufs=4) as xpool,
            ):
                with tc.tile_pool(name="psA", bufs=1, space="PSUM") as psA:
                    wq_r = wq.rearrange("(t p) m -> p t m", p=128)
                    wk_r = wk.rearrange("(t p) m -> p t m", p=128)
                    wv_r = wv.rearrange("(t p) m -> p t m", p=128)
                    xT_r = xT.rearrange("(t p) s -> p t s", p=128)

                    # g=0 weights/x as per-ktile tiles so the very first
                    # matmul only waits on the smallest possible DMA
                    wq0 = [wpool.tile([128, QS], BF16, name=f"wq0_{i}")
                           for i in range(4)]
                    xt0 = [wpool.tile([128, 512], BF16, name=f"xt0_{i}")
                           for i in range(4)]
                    wk0 = [wpool.tile([128, HEAD_DIM], BF16, name=f"wk0_{i}")
                           for i in range(4)]
                    wv0 = [wpool.tile([128, HEAD_DIM], BF16, name=f"wv0_{i}")
                           for i in range(4)]
                    wq_cs, wk_cs, wv_cs = [None], [None], [None]
                    for c in range(1, 8):
                        wq_cs.append(wpool.tile([128, 4, QS], BF16,
                                                name=f"wqc{c}"))
                        wk_cs.append(wpool.tile([128, 4, HEAD_DIM], BF16,
                                                name=f"wkc{c}"))
                        wv_cs.append(wpool.tile([128, 4, HEAD_DIM], BF16,
                                                name=f"wvc{c}"))

                    for sb in range(NSB):
                        ss = slice(sb * 512, (sb + 1) * 512)
                        # k/v accumulators first: phase B's psS reuses the
                        # low PSUM banks, which are evicted earliest at sb3
                        k_ps = psA.tile([128, 512], F32, tag="kps")
                        v_ps = psA.tile([128, 512], F32, tag="vps")
                        q_ps = [psA.tile([128, 512], F32, tag=f"qps{h}",
                                         name=f"qps{h}")
                                for h in range(QH)]

                        for g in range(8):
                            if sb == 0:
                                if g == 0:
                                    # fine-grained startup: all four tensors
                                    # per-ktile, ktile-major across two fast
                                    # queues
                                    for i in range(4):
                                        nc.sync.dma_start(
                                            out=wq0[i], in_=wq_r[:, i, :])
                                        nc.scalar.dma_start(
                                            out=xt0[i], in_=xT_r[:, i, ss])
                                        nc.sync.dma_start(
                                            out=wk0[i], in_=wk_r[:, i, :])
                                        nc.scalar.dma_start(
                                            out=wv0[i], in_=wv_r[:, i, :])
                                    # wk/wv groups up-front on the SWDGE
                                    # queue: per-DMA setup (~1us) overlaps
                                    # the HWDGE weight/x streams
                                    for c in range(1, 8):
                                        cs = slice(c * 4, (c + 1) * 4)
                                        nc.gpsimd.dma_start(
                                            out=wk_cs[c], in_=wk_r[:, cs, :])
                                        nc.gpsimd.dma_start(
                                            out=wv_cs[c], in_=wv_r[:, cs, :])
                                else:
                                    nc.sync.dma_start(
                                        out=wq_cs[g],
                                        in_=wq_r[:, g * 4:(g + 1) * 4, :])
                                if g == 3:
                                    # small constants: off the startup
                                    # critical path
                                    nc.gpsimd.dma_start(out=tri_sb,
                                                        in_=tri[:, :])
                                    nc.gpsimd.dma_start(
                                        out=vext[:, :, 128:129],
                                        in_=onesv[:, :, :])
                            if sb == 2 and g == 0:
                                # wo prefetch while the sync queue is quiet
                                for h in range(QH):
                                    for c in range(2):
                                        nc.sync.dma_start(
                                            out=wo_sb[:, h,
                                                      c * 2048:(c + 1) * 2048],
                                            in_=wo_r[:, h,
                                                     c * 2048:(c + 1) * 2048],
                                        )
                            if not (sb == 0 and g == 0):
                                xt = xpool.tile([128, 4, 512], BF16, tag="xt")
                                nc.scalar.dma_start(
                                    out=xt, in_=xT_r[:, g * 4:(g + 1) * 4, ss]
                                )

                            def wq_ap(i, h):
                                if g == 0:
                                    return wq0[i][:, h * 128:(h + 1) * 128]
                                return wq_cs[g][:, i, h * 128:(h + 1) * 128]

                            def wk_ap(i):
                                return wk0[i] if g == 0 else wk_cs[g][:, i, :]

                            def wv_ap(i):
                                return wv0[i] if g == 0 else wv_cs[g][:, i, :]

                            def x_ap(i):
                                if sb == 0 and g == 0:
                                    return xt0[i]
                                return xt[:, i, :]

                            if g < 7:
                                for i in range(4):
                                    kt = g * 4 + i
                                    st = (kt == 0)
                                    for h in range(QH):
                                        nc.tensor.matmul(
                                            q_ps[h], wq_ap(i, h), x_ap(i),
                                            start=st, stop=False,
                                        )
                                    nc.tensor.matmul(
                                        k_ps, wk_ap(i), x_ap(i),
                                        start=st, stop=False,
                                    )
                                    nc.tensor.matmul(
                                        v_ps, wv_ap(i), x_ap(i),
                                        start=st, stop=False,
                                    )
                            else:
                                # last super-tile: stagger accumulator stops
                                # so evictions overlap the matmul tail.  For
                                # sb3, K/V stop first: phase B's first score
                                # matmuls wait on the k/v/q0/q1 PSUM banks,
                                # so free them in that order.
                                if sb == NSB - 1:
                                    for i in range(4):
                                        nc.tensor.matmul(
                                            k_ps, wk_ap(i), x_ap(i),
                                            start=False, stop=(i == 3),
                                        )
                                    for i in range(4):
                                        nc.tensor.matmul(
                                            v_ps, wv_ap(i), x_ap(i),
                                            start=False, stop=(i == 3),
                                        )
                                    for h in range(QH):
                                        for i in range(4):
                                            nc.tensor.matmul(
                                                q_ps[h], wq_ap(i, h), x_ap(i),
                                                start=False, stop=(i == 3),
                                            )
                                else:
                                    for h in range(QH):
                                        for i in range(4):
                                            nc.tensor.matmul(
                                                q_ps[h], wq_ap(i, h), x_ap(i),
                                                start=False, stop=(i == 3),
                                            )
                                    for i in range(4):
                                        nc.tensor.matmul(
                                            k_ps, wk_ap(i), x_ap(i),
                                            start=False, stop=(i == 3),
                                        )
                                    for i in range(4):
                                        nc.tensor.matmul(
                                            v_ps, wv_ap(i), x_ap(i),
                                            start=False, stop=(i == 3),
                                        )

                        # RoPE tables for this block
                        cos_t = cspool.tile([128, 512], F32, tag="cos")
                        nc.sync.dma_start(out=cos_t, in_=cosT[:, ss])
                        sin_t = cspool.tile([128, 512], F32, tag="sin")
                        nc.sync.dma_start(out=sin_t, in_=sinTs[:, ss])

                        def rope(dst, src_ps):
                            # DVE multiplies the straight view directly from
                            # PSUM (partition-aligned); only the half-rotated
                            # view needs ACT cross-partition copies.
                            vr = ropetmp.tile([128, 512], F32, tag="vr",
                                              name="vr")
                            nc.scalar.copy(vr[0:64, :], src_ps[64:128, :])
                            nc.scalar.copy(vr[64:128, :], src_ps[0:64, :])
                            t = ropetmp.tile([128, 512], F32, tag="t", name="t")
                            u = ropetmp.tile([128, 512], F32, tag="u", name="u")
                            nc.vector.tensor_mul(t, src_ps, cos_t)
                            nc.vector.tensor_mul(u, vr, sin_t)
                            nc.vector.tensor_add(dst, t, u)

                        def emit_vt(sb, vt_sb):
                            # one XBAR op; a 3D out AP yields 4 independent
                            # 128x128 block transposes (verified on HW)
                            nc.sync.dma_start_transpose(
                                out=vext[:, sb * 4:(sb + 1) * 4, 0:128],
                                in_=vt_sb,
                            )

                        if sb == NSB - 1:
                            # evict everything to SBUF fast (DVE, bf16) so
                            # the PSUM banks free at copy pace; RoPE the
                            # copies during phase B (qTs[3]/kTs[3] are first
                            # read at qb=3)
                            nc.vector.tensor_copy(ev3[:, 0, :], k_ps)
                            vt_sb = vtbo.tile([128, 512], BF16, tag="vt")
                            nc.scalar.copy(vt_sb, v_ps)
                            for h in range(QH):
                                nc.vector.tensor_copy(ev3[:, 1 + h, :],
                                                      q_ps[h])
                            # rotated halves via DMA partition swap
                            nc.sync.dma_start(out=vr3[0:64, :, :],
                                              in_=ev3[64:128, :, :])
                            nc.scalar.dma_start(out=vr3[64:128, :, :],
                                                in_=ev3[0:64, :, :])
                            emit_vt(sb, vt_sb)

                            def rope3(dst, idx):
                                t = ropetmp.tile([128, 512], F32, tag="t",
                                                 name="t")
                                u = ropetmp.tile([128, 512], F32, tag="u",
                                                 name="u")
                                nc.gpsimd.tensor_mul(t, ev3[:, idx, :],
                                                     cos_t)
                                nc.gpsimd.tensor_mul(u, vr3[:, idx, :],
                                                     sin_t)
                                nc.gpsimd.tensor_add(dst, t, u)

                            def emit_rope3(sb=sb, rope3=rope3):
                                rope3(kTs[sb], 0)
                                for h in range(QH):
                                    rope3(qTs[sb][:, h, :], 1 + h)
                            rope3_box.append(emit_rope3)
                        else:
                            for h in range(QH):
                                rope(qTs[sb][:, h, :], q_ps[h])
                            rope(kTs[sb], k_ps)
                            vt_sb = vtbo.tile([128, 512], BF16, tag="vt")
                            nc.scalar.copy(vt_sb, v_ps)
                            emit_vt(sb, vt_sb)

                # ---------------- Phase B/C: attention + out projection ----
                with (
                    tc.tile_pool(name="psS", bufs=4, space="PSUM") as psS,
                    tc.tile_pool(name="psO", bufs=1, space="PSUM") as psO,
                ):
                    for qb in range(NSB):
                        n_kb = 4 * qb + 4
                        otT_sb = otpool.tile([128, QH, 512], BF16, tag="ott",
                                             name="ott")
                        def emit_score(h, kb, ess):
                            j = kb - 4 * qb
                            lo = max(0, j) * 128
                            s_ps = psS.tile([128, 512], F32, tag="sps",
                                            name="sps")
                            nc.tensor.matmul(
                                s_ps[:, lo:512],
                                kTs[kb // 4][:, (kb % 4) * 128:
                                             (kb % 4 + 1) * 128],
                                qTs[qb][:, h, lo:512],
                                start=True, stop=True,
                            )
                            es = expp.tile([128, 512], BF16, tag="es",
                                           name="es")
                            nc.scalar.activation(
                                es[:, lo:512], s_ps[:, lo:512],
                                mybir.ActivationFunctionType.Exp,
                                scale=SCALE,
                            )
                            if j >= 0:
                                # causal mask, multiplicative post-exp
                                nc.vector.tensor_mul(
                                    es[:, lo:lo + 128],
                                    es[:, lo:lo + 128],
                                    tri_sb,
                                )
                            ess[kb] = es

                        def mk_drain(h, o_acc, ess, o_sb):
                            def drain(kb):
                                if kb == 0:
                                    for qc in range(QH):
                                        o_acc[qc] = psO.tile(
                                            [128, 129], F32, tag=f"oacc{qc}",
                                            name=f"oacc{qc}",
                                        )
                                for qc in range(4):
                                    if 4 * qb + qc < kb:
                                        continue
                                    stop = (kb == 4 * qb + qc)
                                    nc.tensor.matmul(
                                        o_acc[qc],
                                        ess[kb][:, qc * 128:(qc + 1) * 128],
                                        vext[:, kb, 0:129],
                                        start=(kb == 0), stop=stop,
                                    )
                                    if stop:
                                        dinv = dpool.tile([128, 1], F32,
                                                          tag="dinv",
                                                          name="dinv")
                                        nc.vector.reciprocal(
                                            dinv, o_acc[qc][:, 128:129]
                                        )
                                        nc.vector.tensor_scalar_mul(
                                            o_sb[:, qc, :],
                                            o_acc[qc][:, 0:128],
                                            dinv,
                                        )
                            return drain

                        def emit_ot(h, o_sb):
                            # O -> O^T for the whole head in one XBAR op
                            # (3D out AP -> 4 block transposes)
                            nc.sync.dma_start_transpose(
                                out=otT_sb[:, h, :].rearrange(
                                    "p (four q) -> p four q", four=4),
                                in_=o_sb,
                            )

                        if qb == 0:
                            # short kb runs: all scores/exps first so the
                            # drain burst never waits on ACT latency
                            ess_l = []
                            for h in range(QH):
                                ess = [None] * n_kb
                                for kb in range(n_kb):
                                    emit_score(h, kb, ess)
                                ess_l.append(ess)
                            for h in range(QH):
                                o_acc = [None] * QH
                                o_sb = ospool.tile([128, 4, 128], BF16,
                                                   tag="osb", name="osb")
                                drain = mk_drain(h, o_acc, ess_l[h], o_sb)
                                for kb in range(n_kb):
                                    drain(kb)
                                emit_ot(h, o_sb)
                        else:
                            for h in range(QH):
                                o_acc = [None] * QH
                                ess = [None] * n_kb
                                o_sb = ospool.tile([128, 4, 128], BF16,
                                                   tag="osb", name="osb")
                                drain = mk_drain(h, o_acc, ess, o_sb)
                                for kb in range(n_kb):
                                    emit_score(h, kb, ess)
                                    lag = n_kb if h == 0 else LAG
                                    if kb >= lag:
                                        drain(kb - lag)
                                for kb in range(max(0, n_kb - lag), n_kb):
                                    drain(kb)
                                emit_ot(h, o_sb)

                        if qb == 0:
                            # sb3's RoPE on Pool now: after qb0's tri-mask
                            # muls, long before qb3 reads qTs[3]/kTs[3]
                            for fn in rope3_box:
                                fn()
                            rope3_box.clear()

                        # Phase C for this query block; 2048-wide bf16 stores
                        for qc in range(4):
                            for nb4 in range(2):
                                ob = outev.tile([128, 2048], BF16, tag="ob",
                                                name="ob")
                                for half in range(4):
                                    nb = nb4 * 4 + half
                                    o_ps = psO.tile([128, 512], F32,
                                                    tag=f"oacc{nb % 4}",
                                                    name="ops")
                                    for h in range(QH):
                                        nc.tensor.matmul(
                                            o_ps,
                                            otT_sb[:, h,
                                                   qc * 128:(qc + 1) * 128],
                                            wo_sb[:, h,
                                                  nb * 512:(nb + 1) * 512],
                                            start=(h == 0),
                                            stop=(h == QH - 1),
                                        )
                                    seg = slice(half * 512, (half + 1) * 512)
                                    if half % 2 == 0:
                                        nc.vector.tensor_copy(ob[:, seg], o_ps)
                                    else:
                                        nc.scalar.copy(ob[:, seg], o_ps)
                                rows = slice(qb * 512 + qc * 128,
                                             qb * 512 + (qc + 1) * 128)
                                if qb == 3 and qc == 3:
                                    # kernel tail: halves on both queues so
                                    # the final transfer doesn't serialize
                                    for piece in range(2):
                                        dma_eng = (nc.sync if piece == 0
                                                   else nc.scalar)
                                        cols = slice(
                                            nb4 * 2048 + piece * 1024,
                                            nb4 * 2048 + (piece + 1) * 1024)
                                        dma_eng.dma_start(
                                            out=out[rows, cols],
                                            in_=ob[:, piece * 1024:
                                                   (piece + 1) * 1024],
                                        )
                                else:
                                    dma_eng = (nc.sync if (qc + nb4) % 2 == 0
                                               else nc.scalar)
                                    dma_eng.dma_start(
                                        out=out[rows,
                                                nb4 * 2048:(nb4 + 1) * 2048],
                                        in_=ob,
                                    )
    nc.finalize()
    return nc


_NC_CACHE = {}


def _get_nc():
    if "nc" not in _NC_CACHE:
        _NC_CACHE["nc"] = build_nc()
    return _NC_CACHE["nc"]


def _host_prep(x, cos, sin, mask, wq, wk, wv, wo):
    xT = np.ascontiguousarray(x[0].T.astype(ml_dtypes.bfloat16))
    cosT = np.ascontiguousarray(cos[:, 0, :].T.astype(np.float32))
    sinT = sin[:, 0, :].T.astype(np.float32)
    sinTs = np.ascontiguousarray(
        np.concatenate([-sinT[:64], sinT[64:]], axis=0)
    )
    rr = np.arange(128, dtype=np.int64)[:, None]
    cc = np.arange(128, dtype=np.int64)[None, :]
    tri = np.where(rr > cc, 0.0, 1.0).astype(ml_dtypes.bfloat16)
    onesv = np.ones((128, NKB, 1), dtype=ml_dtypes.bfloat16)

    in_maps = []
    for i in range(N_CORES):
        in_maps.append({
            "xT": xT,
            "wq": np.ascontiguousarray(wq[:, i * QS:(i + 1) * QS].astype(ml_dtypes.bfloat16)),
            "wk": np.ascontiguousarray(wk[:, i * 128:(i + 1) * 128].astype(ml_dtypes.bfloat16)),
            "wv": np.ascontiguousarray(wv[:, i * 128:(i + 1) * 128].astype(ml_dtypes.bfloat16)),
            "wo": np.ascontiguousarray(wo[i * QS:(i + 1) * QS, :].astype(ml_dtypes.bfloat16)),
            "cosT": cosT,
            "sinTs": sinTs,
            "tri": tri,
            "onesv": onesv,
        })
    return in_maps


def kernel(x, cos, sin, mask, wq, wk, wv, wo, _trace=False, _trace_kwargs=None):
    nc = _get_nc()
    in_maps = _host_prep(x, cos, sin, mask, wq, wk, wv, wo)
    res = run_bass_kernel_spmd(
        nc, in_maps, list(range(N_CORES)), trace=_trace,
        **(_trace_kwargs or {}),
    )
    full = np.zeros((SEQ, DIM), dtype=np.float32)
    for i in range(N_CORES):
        full += res.results[i]["out"].astype(np.float32)
    out = full[None, :, :]
    if _trace:
        return out, res
    return out


# revision 12
# speedup vs baseline: 1.0171x; 1.0171x over previous
"""GQA attention (SEQ=2048, DIM=4096, 32 Q heads / 8 KV heads, head_dim=128),
tensor-parallel over heads across 8 NeuronCores.

Each core owns 4 Q heads + 1 KV head: wq/wk/wv split column-wise, wo split
row-wise; each core produces a partial (2048, 4096) output (bf16) that the
host sums (the all-reduce of row-parallel wo).

Per-core kernel:
  A) QKV projections: stream xT (dim-major) blocks; Q^T/K^T/V^T accumulate in
     PSUM over the 4096 contraction; RoPE applied on PSUM eviction; V^T
     transposed back to V-natural (bf16, ones column appended) with XBAR DMA
     transposes (no PE time).  Startup DMAs are per-ktile across queues so
     matmul 0 starts early.  The last seq-block's accumulators are evicted to
     SBUF on DVE so phase B's PSUM pools free immediately; its RoPE (rotated
     halves fetched via DMA partition-swap) overlaps phase B.
  B) Attention per (query-block, head): S^T = K^T_blk.T @ Q^T (keys on
     partitions), exp on ACT (scale=1/sqrt(128) folded in) emitting bf16
     probs, causal triangle applied multiplicatively (0/1 bf16) on the
     diagonal 128-blocks after exp.  AV runs in natural-O orientation:
     lhsT = expS^T 128-col slice, rhs = [V_blk | 1] (129 cols) so the softmax
     denominator accumulates for free in column 128 of PSUM.  Eviction
     normalizes with a per-partition ACT scale (1/D) into bf16; O -> O^T via
     XBAR DMA transpose (no PE time).
  C) out = O^T.T @ wo (all bf16) accumulated over the 4 heads, streamed to
     DRAM as bf16 partials in 2048-wide stores.
"""

import numpy as np
import ml_dtypes

import concourse.bacc as bacc
import concourse.tile as tile
from concourse import mybir
from concourse.bass_utils import run_bass_kernel_spmd

F32 = mybir.dt.float32
BF16 = mybir.dt.bfloat16

DIM = 4096
SEQ = 2048
HEAD_DIM = 128
N_CORES = 8
QH = 4              # q heads per core
QS = QH * HEAD_DIM  # 512: wq column slice per core
NKT = DIM // 128    # 32 contraction tiles
NSB = SEQ // 512    # 4 sequence blocks
NKB = SEQ // 128    # 16 key blocks
SCALE = 1.0 / float(np.sqrt(HEAD_DIM))
LAG = 4             # AV matmuls trail the score stream by LAG key blocks


def build_nc():
    nc = bacc.Bacc(trn_type="TRN2")

    xT = nc.declare_dram_parameter("xT", [DIM, SEQ], BF16, isOutput=False)
    wq = nc.declare_dram_parameter("wq", [DIM, QS], BF16, isOutput=False)
    wk = nc.declare_dram_parameter("wk", [DIM, HEAD_DIM], BF16, isOutput=False)
    wv = nc.declare_dram_parameter("wv", [DIM, HEAD_DIM], BF16, isOutput=False)
    wo = nc.declare_dram_parameter("wo", [QS, DIM], BF16, isOutput=False)
    cosT = nc.declare_dram_parameter("cosT", [HEAD_DIM, SEQ], F32, isOutput=False)
    sinTs = nc.declare_dram_parameter("sinTs", [HEAD_DIM, SEQ], F32, isOutput=False)
    tri = nc.declare_dram_parameter("tri", [128, 128], BF16, isOutput=False)
    onesv = nc.declare_dram_parameter("onesv", [128, NKB, 1], BF16, isOutput=False)
    out = nc.declare_dram_parameter("out", [SEQ, DIM], BF16, isOutput=True)

    with tile.TileContext(nc) as tc:
        with (
            tc.tile_pool(name="persist", bufs=1) as persist,
            tc.tile_pool(name="resid", bufs=1) as resid,
            tc.tile_pool(name="vtbo", bufs=2) as vtbo,
            tc.tile_pool(name="cspool", bufs=2) as cspool,
            tc.tile_pool(name="ropetmp", bufs=2) as ropetmp,
            tc.tile_pool(name="wopool", bufs=1) as wopool,
            tc.tile_pool(name="expp", bufs=18) as expp,
            tc.tile_pool(name="dpool", bufs=6) as dpool,
            tc.tile_pool(name="ospool", bufs=6) as ospool,
            tc.tile_pool(name="otpool", bufs=2) as otpool,
            tc.tile_pool(name="outev", bufs=3) as outev,
        ):
            # resident activations (per-seq-block tiles so cross-phase
            # dependencies stay precise)
            qTs = [resid.tile([128, QH, 512], BF16, name=f"qT{sb}")
                   for sb in range(NSB)]
            kTs = [resid.tile([128, 512], BF16, name=f"kT{sb}")
                   for sb in range(NSB)]
            # V natural (keys, d) in bf16 with a ones column at 128
            # block pitch padded to 144 (16-elem granule) so XBAR DMA
            # transposes land 32B-aligned and never touch the ones column
            vext = resid.tile([128, NKB, 144], BF16)

            tri_sb = persist.tile([128, 128], BF16)
            # sb3 eviction staging (bf16): [k, q0..q3] straight + rotated
            ev3 = persist.tile([128, 5, 512], BF16)
            vr3 = persist.tile([128, 5, 512], BF16)

            rope3_box = []  # sb3 RoPE, emitted after qb0's attention
            wo_sb = wopool.tile([128, QH, DIM], BF16)
            wo_r = wo.rearrange("(h p) n -> p h n", p=128)

            # ---------------- Phase A: projections + RoPE ----------------
            with (
                tc.tile_pool(name="wpool", bufs=1) as wpool,
                tc.tile_pool(name="xpool", bufs=4) as xpool,
            ):
                with tc.tile_pool(name="psA", bufs=1, space="PSUM") as psA:
                    wq_r = wq.rearrange("(t p) m -> p t m", p=128)
                    wk_r = wk.rearrange("(t p) m -> p t m", p=128)
                    wv_r = wv.rearrange("(t p) m -> p t m", p=128)
                    xT_r = xT.rearrange("(t p) s -> p t s", p=128)

                    # g=0 weights/x as per-ktile tiles so the very first
                    # matmul only waits on the smallest possible DMA
                    wq0 = [wpool.tile([128, QS], BF16, name=f"wq0_{i}")
                           for i in range(4)]
                    xt0 = [wpool.tile([128, 512], BF16, name=f"xt0_{i}")
                           for i in range(4)]
                    wk0 = [wpool.tile([128, HEAD_DIM], BF16, name=f"wk0_{i}")
                           for i in range(4)]
                    wv0 = [wpool.tile([128, HEAD_DIM], BF16, name=f"wv0_{i}")
                           for i in range(4)]
                    wq_cs, wk_cs, wv_cs = [None], [None], [None]
                    for c in range(1, 8):
                        wq_cs.append(wpool.tile([128, 4, QS], BF16,
                                                name=f"wqc{c}"))
                        wk_cs.append(wpool.tile([128, 4, HEAD_DIM], BF16,
                                                name=f"wkc{c}"))
                        wv_cs.append(wpool.tile([128, 4, HEAD_DIM], BF16,
                                                name=f"wvc{c}"))

                    for sb in range(NSB):
                        ss = slice(sb * 512, (sb + 1) * 512)
                        # k/v accumulators first: phase B's psS reuses the
                        # low PSUM banks, which are evicted earliest at sb3
                        k_ps = psA.tile([128, 512], F32, tag="kps")
                        v_ps = psA.tile([128, 512], F32, tag="vps")
                        q_ps = [psA.tile([128, 512], F32, tag=f"qps{h}",
                                         name=f"qps{h}")
                                for h in range(QH)]

                        for g in range(8):
                            if sb == 0:
                                if g == 0:
                                    # fine-grained startup: all four tensors
                                    # per-ktile, ktile-major across two fast
                                    # queues
                                    for i in range(4):
                                        nc.sync.dma_start(
                                            out=wq0[i], in_=wq_r[:, i, :])
                                        nc.scalar.dma_start(
                                            out=xt0[i], in_=xT_r[:, i, ss])
                                        nc.sync.dma_start(
                                            out=wk0[i], in_=wk_r[:, i, :])
                                        nc.scalar.dma_start(
                                            out=wv0[i], in_=wv_r[:, i, :])
                                else:
                                    wqe = nc.sync if g % 2 == 1 else nc.scalar
                                    wqe.dma_start(
                                        out=wq_cs[g],
                                        in_=wq_r[:, g * 4:(g + 1) * 4, :])
                                    wqe.dma_start(
                                        out=wk_cs[g],
                                        in_=wk_r[:, g * 4:(g + 1) * 4, :])
                                    wqe.dma_start(
                                        out=wv_cs[g],
                                        in_=wv_r[:, g * 4:(g + 1) * 4, :])
                                if g == 3:
                                    # small constants: off the startup
                                    # critical path
                                    nc.gpsimd.dma_start(out=tri_sb,
                                                        in_=tri[:, :])
                                    nc.gpsimd.dma_start(
                                        out=vext[:, :, 128:129],
                                        in_=onesv[:, :, :])
                            if sb == 2 and g == 0:
                                # wo prefetch while the sync queue is quiet
                                for h in range(QH):
                                    for c in range(2):
                                        nc.sync.dma_start(
                                            out=wo_sb[:, h,
                                                      c * 2048:(c + 1) * 2048],
                                            in_=wo_r[:, h,
                                                     c * 2048:(c + 1) * 2048],
                                        )
                            if not (sb == 0 and g == 0):
                                xt = xpool.tile([128, 4, 512], BF16, tag="xt")
                                if sb == 0:
                                    xe = nc.sync if g % 2 == 0 else nc.scalar
                                else:
                                    xe = nc.scalar
                                xe.dma_start(
                                    out=xt, in_=xT_r[:, g * 4:(g + 1) * 4, ss]
                                )

                            def wq_ap(i, h):
                                if g == 0:
                                    return wq0[i][:, h * 128:(h + 1) * 128]
                                return wq_cs[g][:, i, h * 128:(h + 1) * 128]

                            def wk_ap(i):
                                return wk0[i] if g == 0 else wk_cs[g][:, i, :]

                            def wv_ap(i):
                                return wv0[i] if g == 0 else wv_cs[g][:, i, :]

                            def x_ap(i):
                                if sb == 0 and g == 0:
                                    return xt0[i]
                                return xt[:, i, :]

                            if g < 7:
                                for i in range(4):
                                    kt = g * 4 + i
                                    st = (kt == 0)
                                    for h in range(QH):
                                        nc.tensor.matmul(
                                            q_ps[h], wq_ap(i, h), x_ap(i),
                                            start=st, stop=False,
                                        )
                                    nc.tensor.matmul(
                                        k_ps, wk_ap(i), x_ap(i),
                                        start=st, stop=False,
                                    )
                                    nc.tensor.matmul(
                                        v_ps, wv_ap(i), x_ap(i),
                                        start=st, stop=False,
                                    )
                            else:
                                # last super-tile: stagger accumulator stops
                                # so evictions overlap the matmul tail.  For
                                # sb3, K/V stop first: phase B's first score
                                # matmuls wait on the k/v/q0/q1 PSUM banks,
                                # so free them in that order.
                                if sb == NSB - 1:
                                    for i in range(4):
                                        nc.tensor.matmul(
                                            k_ps, wk_ap(i), x_ap(i),
                                            start=False, stop=(i == 3),
                                        )
                                    for i in range(4):
                                        nc.tensor.matmul(
                                            v_ps, wv_ap(i), x_ap(i),
                                            start=False, stop=(i == 3),
                                        )
                                    for h in range(QH):
                                        for i in range(4):
                                            nc.tensor.matmul(
                                                q_ps[h], wq_ap(i, h), x_ap(i),
                                                start=False, stop=(i == 3),
                                            )
                                else:
                                    for h in range(QH):
                                        for i in range(4):
                                            nc.tensor.matmul(
                                                q_ps[h], wq_ap(i, h), x_ap(i),
                                                start=False, stop=(i == 3),
                                            )
                                    for i in range(4):
                                        nc.tensor.matmul(
                                            k_ps, wk_ap(i), x_ap(i),
                                            start=False, stop=(i == 3),
                                        )
                                    for i in range(4):
                                        nc.tensor.matmul(
                                            v_ps, wv_ap(i), x_ap(i),
                                            start=False, stop=(i == 3),
                                        )

                        # RoPE tables for this block
                        cos_t = cspool.tile([128, 512], F32, tag="cos")
                        nc.sync.dma_start(out=cos_t, in_=cosT[:, ss])
                        sin_t = cspool.tile([128, 512], F32, tag="sin")
                        nc.sync.dma_start(out=sin_t, in_=sinTs[:, ss])

                        def rope(dst, src_ps):
                            # DVE multiplies the straight view directly from
                            # PSUM (partition-aligned); only the half-rotated
                            # view needs ACT cross-partition copies.
                            vr = ropetmp.tile([128, 512], F32, tag="vr",
                                              name="vr")
                            nc.scalar.copy(vr[0:64, :], src_ps[64:128, :])
                            nc.scalar.copy(vr[64:128, :], src_ps[0:64, :])
                            t = ropetmp.tile([128, 512], F32, tag="t", name="t")
                            u = ropetmp.tile([128, 512], F32, tag="u", name="u")
                            nc.vector.tensor_mul(t, src_ps, cos_t)
                            nc.vector.tensor_mul(u, vr, sin_t)
                            nc.vector.tensor_add(dst, t, u)

                        def emit_vt(sb, vt_sb):
                            # one XBAR op; a 3D out AP yields 4 independent
                            # 128x128 block transposes (verified on HW)
                            nc.sync.dma_start_transpose(
                                out=vext[:, sb * 4:(sb + 1) * 4, 0:128],
                                in_=vt_sb,
                            )

                        if sb == NSB - 1:
                            # evict everything to SBUF fast (DVE, bf16) so
                            # the PSUM banks free at copy pace; RoPE the
                            # copies during phase B (qTs[3]/kTs[3] are first
                            # read at qb=3)
                            nc.vector.tensor_copy(ev3[:, 0, :], k_ps)
                            vt_sb = vtbo.tile([128, 512], BF16, tag="vt")
                            nc.scalar.copy(vt_sb, v_ps)
                            for h in range(QH):
                                nc.vector.tensor_copy(ev3[:, 1 + h, :],
                                                      q_ps[h])
                            # rotated halves via DMA partition swap
                            nc.sync.dma_start(out=vr3[0:64, :, :],
                                              in_=ev3[64:128, :, :])
                            nc.scalar.dma_start(out=vr3[64:128, :, :],
                                                in_=ev3[0:64, :, :])
                            emit_vt(sb, vt_sb)

                            def rope3(dst, idx):
                                t = ropetmp.tile([128, 512], F32, tag="t",
                                                 name="t")
                                u = ropetmp.tile([128, 512], F32, tag="u",
                                                 name="u")
                                nc.gpsimd.tensor_mul(t, ev3[:, idx, :],
                                                     cos_t)
                                nc.gpsimd.tensor_mul(u, vr3[:, idx, :],
                                                     sin_t)
                                nc.gpsimd.tensor_add(dst, t, u)

                            def emit_rope3(sb=sb, rope3=rope3):
                                rope3(kTs[sb], 0)
                                for h in range(QH):
                                    rope3(qTs[sb][:, h, :], 1 + h)
                            rope3_box.append(emit_rope3)
                        else:
                            for h in range(QH):
                                rope(qTs[sb][:, h, :], q_ps[h])
                            rope(kTs[sb], k_ps)
                            vt_sb = vtbo.tile([128, 512], BF16, tag="vt")
                            nc.scalar.copy(vt_sb, v_ps)
                            emit_vt(sb, vt_sb)

                # ---------------- Phase B/C: attention + out projection ----
                with (
                    tc.tile_pool(name="psS", bufs=4, space="PSUM") as psS,
                    tc.tile_pool(name="psO", bufs=1, space="PSUM") as psO,
                ):
                    for qb in range(NSB):
                        n_kb = 4 * qb + 4
                        otT_sb = otpool.tile([128, QH, 512], BF16, tag="ott",
                                             name="ott")
                        for h in range(QH):
                            o_acc = [None] * QH
                            ess = [None] * n_kb
                            o_sb = ospool.tile([128, 4, 128], BF16,
                                               tag="osb", name="osb")

                            def drain(kb, h=h, qb=qb, o_acc=o_acc, ess=ess,
                                      o_sb=o_sb):
                                if kb == 0:
                                    for qc in range(QH):
                                        o_acc[qc] = psO.tile(
                                            [128, 129], F32, tag=f"oacc{qc}",
                                            name=f"oacc{qc}",
                                        )
                                for qc in range(4):
                                    if 4 * qb + qc < kb:
                                        continue
                                    stop = (kb == 4 * qb + qc)
                                    nc.tensor.matmul(
                                        o_acc[qc],
                                        ess[kb][:, qc * 128:(qc + 1) * 128],
                                        vext[:, kb, 0:129],
                                        start=(kb == 0), stop=stop,
                                    )
                                    if stop:
                                        dinv = dpool.tile([128, 1], F32,
                                                          tag="dinv",
                                                          name="dinv")
                                        nc.vector.reciprocal(
                                            dinv, o_acc[qc][:, 128:129]
                                        )
                                        nc.vector.tensor_scalar_mul(
                                            o_sb[:, qc, :],
                                            o_acc[qc][:, 0:128],
                                            dinv,
                                        )

                            for kb in range(n_kb):
                                j = kb - 4 * qb
                                lo = max(0, j) * 128
                                s_ps = psS.tile([128, 512], F32, tag="sps",
                                                name="sps")
                                nc.tensor.matmul(
                                    s_ps[:, lo:512],
                                    kTs[kb // 4][:, (kb % 4) * 128:
                                                 (kb % 4 + 1) * 128],
                                    qTs[qb][:, h, lo:512],
                                    start=True, stop=True,
                                )
                                es = expp.tile([128, 512], BF16, tag="es",
                                               name="es")
                                nc.scalar.activation(
                                    es[:, lo:512], s_ps[:, lo:512],
                                    mybir.ActivationFunctionType.Exp,
                                    scale=SCALE,
                                )
                                if j >= 0:
                                    # causal mask, multiplicative post-exp
                                    nc.vector.tensor_mul(
                                        es[:, lo:lo + 128],
                                        es[:, lo:lo + 128],
                                        tri_sb,
                                    )
                                ess[kb] = es
                                lag = n_kb if h == 0 else LAG
                                if kb >= lag:
                                    drain(kb - lag)
                            for kb in range(max(0, n_kb - lag), n_kb):
                                drain(kb)
                            # O -> O^T for the whole head in one XBAR op
                            # (3D out AP -> 4 block transposes)
                            nc.sync.dma_start_transpose(
                                out=otT_sb[:, h, :].rearrange(
                                    "p (four q) -> p four q", four=4),
                                in_=o_sb,
                            )

                        if qb == 0:
                            # sb3's RoPE on Pool now: after qb0's tri-mask
                            # muls, long before qb3 reads qTs[3]/kTs[3]
                            for fn in rope3_box:
                                fn()
                            rope3_box.clear()

                        # Phase C for this query block; 2048-wide bf16 stores
                        for qc in range(4):
                            for nb4 in range(2):
                                ob = outev.tile([128, 2048], BF16, tag="ob",
                                                name="ob")
                                for half in range(4):
                                    nb = nb4 * 4 + half
                                    o_ps = psO.tile([128, 512], F32,
                                                    tag=f"oacc{nb % 4}",
                                                    name="ops")
                                    for h in range(QH):
                                        nc.tensor.matmul(
                                            o_ps,
                                            otT_sb[:, h,
                                                   qc * 128:(qc + 1) * 128],
                                            wo_sb[:, h,
                                                  nb * 512:(nb + 1) * 512],
                                            start=(h == 0),
                                            stop=(h == QH - 1),
                                        )
                                    seg = slice(half * 512, (half + 1) * 512)
                                    if half % 2 == 0:
                                        nc.vector.tensor_copy(ob[:, seg], o_ps)
                                    else:
                                        nc.scalar.copy(ob[:, seg], o_ps)
                                rows = slice(qb * 512 + qc * 128,
                                             qb * 512 + (qc + 1) * 128)
                                if qb == 3 and qc == 3:
                                    # kernel tail: halves on both queues so
                                    # the final transfer doesn't serialize
                                    for piece in range(2):
                                        dma_eng = (nc.sync if piece == 0
                                                   else nc.scalar)
                                        cols = slice(
                                            nb4 * 2048 + piece * 1024,
                                            nb4 * 2048 + (piece + 1) * 1024)
                                        dma_eng.dma_start(
                                            out=out[rows, cols],
                                            in_=ob[:, piece * 1024:
                                                   (piece + 1) * 1024],
                                        )
                                else:
                                    dma_eng = (nc.sync if (qc + nb4) % 2 == 0
                                               else nc.scalar)
                                    dma_eng.dma_start(
                                        out=out[rows,
                                                nb4 * 2048:(nb4 + 1) * 2048],
                                        in_=ob,
                                    )
    nc.finalize()
    return nc


_NC_CACHE = {}


def _get_nc():
    if "nc" not in _NC_CACHE:
        _NC_CACHE["nc"] = build_nc()
    return _NC_CACHE["nc"]


def _host_prep(x, cos, sin, mask, wq, wk, wv, wo):
    xT = np.ascontiguousarray(x[0].T.astype(ml_dtypes.bfloat16))
    cosT = np.ascontiguousarray(cos[:, 0, :].T.astype(np.float32))
    sinT = sin[:, 0, :].T.astype(np.float32)
    sinTs = np.ascontiguousarray(
        np.concatenate([-sinT[:64], sinT[64:]], axis=0)
    )
    rr = np.arange(128, dtype=np.int64)[:, None]
    cc = np.arange(128, dtype=np.int64)[None, :]
    tri = np.where(rr > cc, 0.0, 1.0).astype(ml_dtypes.bfloat16)
    onesv = np.ones((128, NKB, 1), dtype=ml_dtypes.bfloat16)

    in_maps = []
    for i in range(N_CORES):
        in_maps.append({
            "xT": xT,
            "wq": np.ascontiguousarray(wq[:, i * QS:(i + 1) * QS].astype(ml_dtypes.bfloat16)),
            "wk": np.ascontiguousarray(wk[:, i * 128:(i + 1) * 128].astype(ml_dtypes.bfloat16)),
            "wv": np.ascontiguousarray(wv[:, i * 128:(i + 1) * 128].astype(ml_dtypes.bfloat16)),
            "wo": np.ascontiguousarray(wo[i * QS:(i + 1) * QS, :].astype(ml_dtypes.bfloat16)),
            "cosT": cosT,
            "sinTs": sinTs,
            "tri": tri,
            "onesv": onesv,
        })
    return in_maps


def kernel(x, cos, sin, mask, wq, wk, wv, wo, _trace=False, _trace_kwargs=None):
    nc = _get_nc()
    in_maps = _host_prep(x, cos, sin, mask, wq, wk, wv, wo)
    res = run_bass_kernel_spmd(
        nc, in_maps, list(range(N_CORES)), trace=_trace,
        **(_trace_kwargs or {}),
    )
    full = np.zeros((SEQ, DIM), dtype=np.float32)
    for i in range(N_CORES):
        full += res.results[i]["out"].astype(np.float32)
    out = full[None, :, :]
    if _trace:
        return out, res
    return out
